# revision 55
# baseline (speedup 1.0000x reference)
"""AFT-Full (Attention Free Transformer, full position bias) on 8 TRN2
NeuronCores.

Problem (per reference.py):
    x [16, 2048, 512] f32, Wq/Wk/Wv [512, 512], bq/bk/bv [512],
    pos_bias [2048, 2048]
    q = x@Wq+bq; k = x@Wk+bk; v = x@Wv+bv
    out[b,i,d] = sigmoid(q)[b,i,d]
                 * sum_j exp(k+bias[i,j])*v / sum_j exp(k+bias[i,j])

Sharding: pure data-parallel over the batch (16 batches -> 2 per core).
Every core holds a replica of the weights and pos_bias; zero cross-core
communication.

Numerics / speed strategy (~216us HW, rel err ~0.9% vs the 2e-2 gate;
v1 baseline was ~260us):
  - All input-only transforms run on the HOST: x / weights pre-cast to
    bf16 (and fp8 for the q path), and the shifted position-bias
    operand
        u = exp(pos_bias) - 1            (fp8e4, scaled by 256)
    is quantized on the host in the exact [jl, jb, i] SBUF layout the
    stage-2 matmuls consume.  This removes ~18MB of f32 DMA and the
    entire exp/quantize pipeline (~70us of ScalarE + ~25us of DVE) that
    made stage 1 DMA/ACT-bound in the v1 baseline.
  - Stage 1: v/k projections in bf16 (their precision feeds the exact
    colsum term and must stay ~bf16: fp8 there costs ~3.6% output
    error because the output is itself a 1/sqrt(N)-scale weighted
    mean).  The q projection only feeds a sigmoid gate, so it runs in
    fp8e4 DoubleRow with host-scaled operands (x*16, Wq*8, undone by
    exp(-q/128)) -- 10 matmuls/tile instead of 12.
  - Stage 2 (the [N,N] x [N,2BD] num/den contraction) runs in fp8e4
    with the DoubleRow perf mode via the shifted decomposition
        num[i,d] = sum_j ev[j,d]  +  sum_j u[i,j] ev[j,d]
        den[i,d] = sum_j ek[j,d]  +  sum_j u[i,j] ek[j,d]
    The i-independent colsum terms carry ~90% of the magnitude and are
    accumulated exactly in f32; only the small u-contraction runs in
    fp8 (u RMS ~0.1 vs eb ~1.0).  u is scaled by 256 and [ev|ek] by 4;
    the epilogue divides the PSUM result by 1024 and adds the colsums
    back.  Its 512 DR matmuls issue at the 216ns N=512 stream rate --
    the fp8 roofline for this contraction.
  - The fp8 ek operand comes straight out of the ScalarE exp:
    exp(k + ln 4) = 4*exp(k), so no separate quantize op for the ek
    half (and no DVE read of kv that would collide with the gpsimd
    colsum ADD -- concurrent same-tile SBUF readers on different
    engines run ~5x slower).
  - Stage 2 accumulates each batch in its own 2-bank PSUM group
    ([num_b|den_b]); per-group epilogue chains stay on the DVE with
    emp1 staged through an idle PSUM bank (a 3-SBUF-stream f32 op runs
    ~1380ns vs ~600ns with one PSUM operand).
  - sigmoid(q)*num/den = num / (den*(1+exp(-q))), so the scalar engine
    only ever evaluates Exp.
"""

import math
from contextlib import ExitStack

import ml_dtypes
import numpy as np

import concourse.bacc as bacc
import concourse.mybir as mybir
import concourse.tile as tile
from concourse.bass_isa import ReduceOp
from concourse.bass_utils import run_bass_kernel_spmd

F32 = mybir.dt.float32
BF16 = mybir.dt.bfloat16
F8 = mybir.dt.float8e4
P = 128

N_CORES = 8
BATCH = 16
N = 2048
D_MODEL = 512

# mybir float8e4 is IEEE-style e4m3: max finite 240, overflow -> inf.
# Keep scaled maxima comfortably below 240 (|u|<~0.75, |ev|<~30, ek<~12).
U_SCALE = 256.0     # fp8 scale for u = exp(pos_bias) - 1
KV_SCALE = 4.0      # fp8 scale for [ev|ek]
INV_SCALE = 1.0 / (U_SCALE * KV_SCALE)
LN_KV_SCALE = math.log(KV_SCALE)


def _install_axon_ntff_shim():
    """Make run_bass_kernel_spmd(trace=True) work when the image's antenv
    lacks axon_hooks (the hook degrades tracing otherwise).  No-op when a
    real antenv.axon_hooks is importable."""
    import sys
    import types

    try:
        import antenv.axon_hooks  # noqa: F401
        return
    except ImportError:
        pass
    try:
        from trn_agent_boot.trn_boot import _ntff_profile_via_ctypes
        hook = _ntff_profile_via_ctypes("/opt/axon/libaxon_pjrt.so")
    except Exception:
        hook = None
    mod = types.ModuleType("antenv.axon_hooks")
    mod.get_axon_ntff_profile_hook = lambda: hook
    mod.set_axon_ntff_profile_hook = lambda h: None
    sys.modules["antenv.axon_hooks"] = mod

    import concourse.bass_utils as bass_utils
    _orig_upload = bass_utils.upload_artifacts

    def _safe_upload(tmpdir):
        try:
            return _orig_upload(tmpdir)
        except Exception:
            return tmpdir
    bass_utils.upload_artifacts = _safe_upload


def build_aft(B=2, N=2048, D=512, n_cores=8, use_bias=False):
    NT = N // P          # row tiles per batch (t / j / i tiles)
    DB = D // P          # d_model blocks of 128 (contraction for projections)
    QKV = 3 * D
    C2 = 2 * B * D       # [ev_b0|ek_b0|ev_b1|ek_b1] column layout
    XW = 8 * P           # x DMA batching: eight t-tiles per transfer
    Exp = mybir.ActivationFunctionType.Exp
    Ident = mybir.ActivationFunctionType.Identity
    Alu = mybir.AluOpType
    DR = mybir.MatmulPerfMode.DoubleRow

    nc = bacc.Bacc("TRN2", target_bir_lowering=False, debug=False,
                   num_devices=n_cores)

    xT_e = nc.dram_tensor("xT", [B, D, N], BF16, kind="ExternalInput")
    x8_e = nc.dram_tensor("x8", [B, D, N], F8, kind="ExternalInput")
    # only v|k in bf16 -- the q projection reads the fp8 wq8 instead, so
    # shipping bf16 Wq would waste 0.5MB of the critical head DMA window
    w_e = nc.dram_tensor("wvk", [D, 2 * D], BF16, kind="ExternalInput")
    wq8_e = nc.dram_tensor("wq8", [P, DB * D], F8, kind="ExternalInput")
    u8_e = nc.dram_tensor("u8", [P, NT, N], F8, kind="ExternalInput")
    if use_bias:
        b_e = nc.dram_tensor("bvkq", [1, QKV], BF16, kind="ExternalInput")
    out_e = nc.dram_tensor("out", [B, N, D], F32, kind="ExternalOutput")

    with tile.TileContext(nc) as tc, ExitStack() as ctx:
        persist = ctx.enter_context(tc.tile_pool(name="persist", bufs=1))
        # psA: [P,1024] granules (2 PSUM banks): stage-1 v|k, stage-2 num|den
        # psB: [P,512] granules (1 bank): stage-1 q, warmup
        psA = ctx.enter_context(tc.tile_pool(name="psA", bufs=3, space="PSUM"))
        psB = ctx.enter_context(tc.tile_pool(name="psB", bufs=2, space="PSUM"))

        # ---- persistent SBUF tensors ----
        u8_sb = persist.tile([P, NT, N], F8)             # 256*(exp(pbT)-1)
        ekv_sb = persist.tile([P, NT, C2], F8)           # 4*[ev|ek] per batch
        q_sb = persist.tile([P, B * NT, D], BF16)        # exp(-q)
        acc_sb = persist.tile([P, C2], F32)              # colsum accumulator
        cs_sb = persist.tile([P, C2], F32)               # all-reduced colsums
        cln4 = persist.tile([P, 1], F32)                 # bias AP: ln(KV_SCALE)
        nc.gpsimd.memset(cln4[:], LN_KV_SCALE)

        with ExitStack() as s1:
            wpool = s1.enter_context(tc.tile_pool(name="wpool", bufs=1))
            kvpool = s1.enter_context(tc.tile_pool(name="kvpool", bufs=6))

            # ---- input DMAs ----
            # x (bf16, for v/k) and x8 (fp8, for the DoubleRow q matmul)
            # live in persistent SBUF tiles.  The leading pieces are small
            # so the first t-tile arrives with minimal transfers; the bulk
            # pieces are paced into the tile loop with a ~4-tile lead so
            # they never crowd the head of the DMA queues (the queues
            # spool up only ~8us after kernel start).
            x_sb = persist.tile([P, DB, B * N], BF16)
            x8_sb = persist.tile([P, DB, B * N], F8)
            xT_r = xT_e.ap().rearrange("b (db p) n -> b p db n", p=P)
            x8_r = x8_e.ap().rearrange("b (db p) n -> b p db n", p=P)

            def emit_x_dma(xb, xt0, xw):
                s0, s1 = (xb * NT + xt0) * P, (xb * NT + xt0 + xw) * P
                nc.sync.dma_start(x_sb[:, :, s0:s1],
                                  xT_r[xb, :, :, xt0 * P:(xt0 + xw) * P])
                nc.sync.dma_start(x8_sb[:, :, s0:s1],
                                  x8_r[xb, :, :, xt0 * P:(xt0 + xw) * P])

            # head pieces: tiles 0-5 of batch 0, interleaved with the
            # weights in consumption order ([v|k] split per d-block so
            # tile 0's first matmuls never wait on a 384KB transfer)
            w_sb = wpool.tile([P, DB, 2 * D], BF16)
            w_r = w_e.ap().rearrange("(db p) c -> db p c", p=P)
            wq_sb = wpool.tile([P, DB, D], F8)
            emit_x_dma(0, 0, 1)
            nc.sync.dma_start(w_sb[:, 0, :D], w_r[0][:, :D])
            emit_x_dma(0, 1, 1)
            nc.sync.dma_start(w_sb[:, 0, D:], w_r[0][:, D:])
            nc.sync.dma_start(wq_sb[:, :, :],
                              wq8_e.ap().rearrange("p (db c) -> p db c", c=D))
            # tiles 2 and 3 as separate 128KB pieces: a combined 256KB
            # piece lands ~3us later than the single-tile ones and was
            # the first head stall
            emit_x_dma(0, 2, 1)
            emit_x_dma(0, 3, 1)
            nc.sync.dma_start(w_sb[:, 1, :D], w_r[1][:, :D])
            nc.sync.dma_start(w_sb[:, 1, D:], w_r[1][:, D:])
            emit_x_dma(0, 4, 2)
            for db in range(2, DB):
                nc.sync.dma_start(w_sb[:, db, :D], w_r[db][:, :D])
                nc.sync.dma_start(w_sb[:, db, D:], w_r[db][:, D:])
            if use_bias:
                bias_sb = wpool.tile([1, QKV], BF16)
                nc.sync.dma_start(bias_sb[:], b_e.ap())
                ones_sb = wpool.tile([1, P], BF16)
                nc.vector.memset(ones_sb[:], 1.0)

            # bulk pieces: 2-tile transfers (a 4-tile piece takes too long
            # on its pair of queues and arrives after its consumer).  The
            # early pieces are spread one-per-step (a step-0 burst would
            # starve the head tiles' transfers); later ones keep an
            # 8-tile lead over consumption.
            x_sched = {0: [(0, 6, 2)], 1: [(0, 8, 2)], 2: [(0, 10, 2)],
                       4: [(0, 12, 2)], 6: [(0, 14, 2)]}
            for gidx in range(2 * NT // 2, 2 * NT, 2):
                x_sched.setdefault(gidx - 8, []).append(
                    (gidx // NT, gidx % NT, 2))

            # ---- PE warmup ----
            # The PE clock-gate (HAM) starts at 1.2GHz and releases to
            # 2.4GHz only after ~3.4us of sustained activity.  Issue
            # throwaway matmuls on a memset tile so the clock is warm when
            # the first real projection's x tile lands (~12us in).
            wa = wpool.tile([P, 64], BF16)
            nc.gpsimd.memset(wa[:], 0.0)
            nc.gpsimd.memset(acc_sb[:], 0.0)
            wps = psB.tile([P, D], F32, tag="psB", name="wps")
            for w_i in range(95):
                nc.tensor.matmul(wps[0:64, 0:64], wa[:, 0:64], wa[:],
                                 start=(w_i == 0), stop=(w_i == 94))

            # u8 arrives in four paced 1MB transfers (fp8, host-quantized)
            def emit_u8_dma(part):
                nc.sync.dma_start(u8_sb[:, 4 * part:4 * part + 4, :],
                                  u8_e.ap()[:, 4 * part:4 * part + 4, :])

            # ---- stage 1: projections v/k/q + exp epilogue ----
            step = 0
            for b in range(B):
                for t in range(NT):
                    s = b * NT + t
                    for piece in x_sched.get(step, ()):
                        emit_x_dma(*piece)
                    # pace the u8 stream into the middle of stage 1
                    # (stage 2 only reads u8 ~60us later)
                    if step in (4, 10, 16, 22):
                        emit_u8_dma((step - 4) // 6)

                    ps = psA.tile([P, 2 * D], F32, tag="psA")
                    pq = psB.tile([P, D], F32, tag="psB")
                    for db in range(DB):
                        xt = x_sb[:, db, s * P:(s + 1) * P]
                        st, sp = (db == 0), (db == DB - 1 and not use_bias)
                        nc.tensor.matmul(ps[:, 0:D], xt, w_sb[:, db, 0:D],
                                         start=st, stop=sp)
                        nc.tensor.matmul(ps[:, D:2 * D], xt,
                                         w_sb[:, db, D:2 * D],
                                         start=st, stop=sp)
                    # q = x8 @ wq8 in fp8 DoubleRow (2 passes of 256 rows);
                    # operands are host-scaled by 16 and 8, undone in the
                    # exp(-q/128) epilogue below
                    for h in range(2):
                        nc.tensor.matmul(pq[:, :],
                                         x8_sb[:, 2 * h:2 * h + 2,
                                               s * P:(s + 1) * P],
                                         wq_sb[:, 2 * h:2 * h + 2, :],
                                         start=(h == 0),
                                         stop=(h == 1 and not use_bias),
                                         perf_mode=DR)
                    if use_bias:
                        nc.tensor.matmul(ps[:, 0:D], ones_sb[:, :],
                                         bias_sb[:, 0:D],
                                         start=False, stop=True)
                        nc.tensor.matmul(ps[:, D:2 * D], ones_sb[:, :],
                                         bias_sb[:, D:2 * D],
                                         start=False, stop=True)
                        # q operands are scaled by 128; scale bq to match
                        nc.tensor.matmul(pq[:, :], ones_sb[:, :],
                                         bias_sb[:, 2 * D:],
                                         start=False, stop=True)

                    col = b * 2 * D
                    # bf16 kv = [ev|ek] feeds the f32 colsum + the ev mul
                    kv = kvpool.tile([P, 2 * D], BF16, tag="kv")
                    nc.scalar.activation(kv[:, D:2 * D], ps[:, D:2 * D], Exp)
                    # fp8 ek = 4*exp(k) always on the scalar engine, reading
                    # PSUM: a DVE quantize of kv[D:2D] would overlap the
                    # gpsimd colsum ADD's read of the same kv tile, and that
                    # same-tile SBUF contention slows the DVE op ~5x
                    nc.scalar.activation(
                        ekv_sb[:, t, col + D:col + 2 * D],
                        ps[:, D:2 * D], Exp, bias=cln4[:])
                    # em = exp(-q/128): epilogue uses sigmoid(q)*num/den =
                    # num / (den * (1 + exp(-q)))
                    nc.scalar.activation(q_sb[:, b * NT + t, :], pq[:, :],
                                         Exp, scale=-1.0 / 128.0)
                    nc.vector.tensor_mul(kv[:, 0:D], kv[:, D:2 * D],
                                         ps[:, 0:D])
                    # fp8 ev = 4*ev.  On b1 the DVE also carries the colsum
                    # chain and sits at ~97% occupancy, so alternate b1
                    # tiles hand this quantize to the scalar engine (which
                    # has ~15% slack there; gpsimd is idle on b1 so there
                    # is no same-tile kv contention from it)
                    if b == 1 and t % 2 == 1:
                        nc.scalar.activation(ekv_sb[:, t, col:col + D],
                                             kv[:, 0:D], Ident,
                                             scale=KV_SCALE)
                    else:
                        nc.vector.tensor_scalar_mul(
                            ekv_sb[:, t, col:col + D], kv[:, 0:D], KV_SCALE)
                    # f32 colsum accumulation: serial per-batch chain kept
                    # on ONE engine each (cross-engine chains head-block
                    # the strict FIFOs; gpsimd also reads fp8 ~10x slower
                    # than bf16, so both chains read the bf16 kv):
                    # b0 on gpsimd, b1 on the DVE
                    acc_eng = nc.gpsimd if b == 0 else nc.vector
                    acc_eng.tensor_add(acc_sb[:, col:col + 2 * D],
                                       acc_sb[:, col:col + 2 * D], kv[:])
                    if s == NT:
                        # b0 colsum all-reduce overlapped with b1 stage 1;
                        # its entire input chain also lived on gpsimd, so
                        # this never waits on another engine
                        nc.gpsimd.partition_all_reduce(
                            cs_sb[:, 0:2 * D], acc_sb[:, 0:2 * D],
                            P, ReduceOp.add)
                    step += 1

            nc.gpsimd.partition_all_reduce(
                cs_sb[:, 2 * D:], acc_sb[:, 2 * D:], P, ReduceOp.add)

        # ---- stage 2: num/den contraction over j + epilogue ----
        epi = ctx.enter_context(tc.tile_pool(name="epi", bufs=3))

        for i in range(NT):
            for g in range(B):        # per-batch PSUM group [num_b|den_b]
                # emp1 = 1 + exp(-q) in f32, written into a PSUM bank: the
                # t1 multiply below then reads one PSUM + one SBUF operand,
                # which runs at ~600ns instead of the ~1380ns an all-SBUF
                # f32 three-stream op costs on the DVE.  psB's banks are
                # idle in stage 2.  Emitted before the matmul group so it
                # never sits in the post-matmul chain.
                emp1 = psB.tile([P, D], F32, tag="psB")
                nc.scalar.activation(emp1[:], q_sb[:, g * NT + i, :],
                                     Ident, bias=1.0)

                ps = psA.tile([P, 2 * D], F32, tag="psA")
                for jb2 in range(NT // 2):
                    lhsT = u8_sb[:, 2 * jb2:2 * jb2 + 2, i * P:(i + 1) * P]
                    st, sp = (jb2 == 0), (jb2 == NT // 2 - 1)
                    nc.tensor.matmul(
                        ps[:, 0:D], lhsT,
                        ekv_sb[:, 2 * jb2:2 * jb2 + 2,
                               2 * g * D:(2 * g + 1) * D],
                        start=st, stop=sp, perf_mode=DR)
                    nc.tensor.matmul(
                        ps[:, D:2 * D], lhsT,
                        ekv_sb[:, 2 * jb2:2 * jb2 + 2,
                               (2 * g + 1) * D:(2 * g + 2) * D],
                        start=st, stop=sp, perf_mode=DR)

                col = g * 2 * D
                # num/den = psum/1024 + colsum   (the exact shifted term).
                # The whole per-group chain stays on the DVE: a gpsimd hop
                # inside the chain head-blocks the DVE FIFO (gpsimd muls
                # are ~1.4us) and stalls PSUM recycling.  Only the final
                # o=num*r multiply -- which feeds nothing but the out DMA
                # -- goes to gpsimd (DVE for the last group, whose chain
                # is the exposed kernel tail).  The out store needs no
                # splitting: a dma_start is descriptor-split across all 16
                # queues and moves 256KB in ~2.5us.
                last = (i == NT - 1 and g == B - 1)
                den = epi.tile([P, D], F32, tag="den")
                nc.vector.scalar_tensor_tensor(
                    den[:], ps[:, D:2 * D], INV_SCALE,
                    cs_sb[:, col + D:col + 2 * D],
                    Alu.mult, Alu.add)
                num = epi.tile([P, D], F32, tag="num")
                nc.vector.scalar_tensor_tensor(
                    num[:], ps[:, 0:D], INV_SCALE, cs_sb[:, col:col + D],
                    Alu.mult, Alu.add)
                t1 = epi.tile([P, D], F32, tag="t1")
                nc.vector.tensor_mul(t1[:], emp1[:], den[:])
                r = epi.tile([P, D], F32, tag="r")
                nc.vector.reciprocal_approx_fast(r[:], t1[:])
                o = epi.tile([P, D], F32, tag="o")
                o_eng = nc.vector if last else nc.gpsimd
                o_eng.tensor_mul(o[:], num[:], r[:])
                nc.sync.dma_start(out_e.ap()[g, i * P:(i + 1) * P], o[:])

    nc.compile()
    return nc


_NC_CACHE = {}


def _get_nc(use_bias):
    key = bool(use_bias)
    if key not in _NC_CACHE:
        _NC_CACHE[key] = build_aft(B=BATCH // N_CORES, N=N, D=D_MODEL,
                                   n_cores=N_CORES, use_bias=key)
    return _NC_CACHE[key]


def make_in_maps(x, Wq, bq, Wk, bk, Wv, bv, pos_bias, use_bias):
    """Host-side prep: bf16 casts + the fp8 shifted pos-bias operand in
    stage-2 SBUF layout [jl, jb, i]."""
    NT = N // P
    DB = D_MODEL // P
    Bc = BATCH // N_CORES
    wvk = np.concatenate([Wv, Wk], axis=1).astype(ml_dtypes.bfloat16)
    # q runs in fp8 DoubleRow: host-scale x by 16 and Wq by 8 to lift the
    # operands out of the e4m3 subnormal range; exp(-q/128) undoes it
    wq8 = np.clip(8.0 * Wq, -240, 240).astype(ml_dtypes.float8_e4m3)
    wq8 = np.ascontiguousarray(
        wq8.reshape(DB, P, D_MODEL).transpose(1, 0, 2)).reshape(P, DB * D_MODEL)
    u = U_SCALE * np.expm1(pos_bias.astype(np.float64))       # [i, j]
    u8 = np.clip(u.T, -240.0, 240.0).astype(ml_dtypes.float8_e4m3)  # [j, i]
    u8 = np.ascontiguousarray(
        u8.reshape(NT, P, N).transpose(1, 0, 2))              # [jl, jb, i]
    in_maps = []
    for c in range(N_CORES):
        xT = np.ascontiguousarray(x[c * Bc:(c + 1) * Bc].transpose(0, 2, 1))
        im = {
            "xT": xT.astype(ml_dtypes.bfloat16),
            "x8": np.clip(16.0 * xT, -240, 240).astype(ml_dtypes.float8_e4m3),
            "wvk": wvk,
            "wq8": wq8,
            "u8": u8,
        }
        if use_bias:
            im["bvkq"] = np.concatenate(
                [bv, bk, 128.0 * bq])[None, :].astype(ml_dtypes.bfloat16)
        in_maps.append(im)
    return in_maps


def kernel(x, Wq, bq, Wk, bk, Wv, bv, pos_bias):
    x = np.asarray(x, dtype=np.float32)
    Wq = np.asarray(Wq, dtype=np.float32)
    Wk = np.asarray(Wk, dtype=np.float32)
    Wv = np.asarray(Wv, dtype=np.float32)
    bq = np.asarray(bq, dtype=np.float32)
    bk = np.asarray(bk, dtype=np.float32)
    bv = np.asarray(bv, dtype=np.float32)
    pos_bias = np.asarray(pos_bias, dtype=np.float32)
    assert x.shape == (BATCH, N, D_MODEL)
    assert pos_bias.shape == (N, N)

    _install_axon_ntff_shim()

    use_bias = bool(np.any(bq) or np.any(bk) or np.any(bv))
    nc = _get_nc(use_bias)
    in_maps = make_in_maps(x, Wq, bq, Wk, bk, Wv, bv, pos_bias, use_bias)
    res = run_bass_kernel_spmd(nc, in_maps, core_ids=list(range(N_CORES)))
    out = np.concatenate([res.results[c]["out"] for c in range(N_CORES)],
                         axis=0)
    return out.astype(np.float32, copy=False)


# revision 56
# speedup vs baseline: 1.0093x; 1.0093x over previous
"""AFT-Full (Attention Free Transformer, full position bias) on 8 TRN2
NeuronCores.

Problem (per reference.py):
    x [16, 2048, 512] f32, Wq/Wk/Wv [512, 512], bq/bk/bv [512],
    pos_bias [2048, 2048]
    q = x@Wq+bq; k = x@Wk+bk; v = x@Wv+bv
    out[b,i,d] = sigmoid(q)[b,i,d]
                 * sum_j exp(k+bias[i,j])*v / sum_j exp(k+bias[i,j])

Sharding: pure data-parallel over the batch (16 batches -> 2 per core).
Every core holds a replica of the weights and pos_bias; zero cross-core
communication.

Numerics / speed strategy (~216us HW, rel err ~0.9% vs the 2e-2 gate;
v1 baseline was ~260us):
  - All input-only transforms run on the HOST: x / weights pre-cast to
    bf16 (and fp8 for the q path), and the shifted position-bias
    operand
        u = exp(pos_bias) - 1            (fp8e4, scaled by 256)
    is quantized on the host in the exact [jl, jb, i] SBUF layout the
    stage-2 matmuls consume.  This removes ~18MB of f32 DMA and the
    entire exp/quantize pipeline (~70us of ScalarE + ~25us of DVE) that
    made stage 1 DMA/ACT-bound in the v1 baseline.
  - Stage 1: v/k projections in bf16 (their precision feeds the exact
    colsum term and must stay ~bf16: fp8 there costs ~3.6% output
    error because the output is itself a 1/sqrt(N)-scale weighted
    mean).  The q projection only feeds a sigmoid gate, so it runs in
    fp8e4 DoubleRow with host-scaled operands (x*16, Wq*8, undone by
    exp(-q/128)) -- 10 matmuls/tile instead of 12.
  - Stage 2 (the [N,N] x [N,2BD] num/den contraction) runs in fp8e4
    with the DoubleRow perf mode via the shifted decomposition
        num[i,d] = sum_j ev[j,d]  +  sum_j u[i,j] ev[j,d]
        den[i,d] = sum_j ek[j,d]  +  sum_j u[i,j] ek[j,d]
    The i-independent colsum terms carry ~90% of the magnitude and are
    accumulated exactly in f32; only the small u-contraction runs in
    fp8 (u RMS ~0.1 vs eb ~1.0).  u is scaled by 256 and [ev|ek] by 4;
    the epilogue divides the PSUM result by 1024 and adds the colsums
    back.  Its 512 DR matmuls issue at the 216ns N=512 stream rate --
    the fp8 roofline for this contraction.
  - The fp8 ek operand comes straight out of the ScalarE exp:
    exp(k + ln 4) = 4*exp(k), so no separate quantize op for the ek
    half (and no DVE read of kv that would collide with the gpsimd
    colsum ADD -- concurrent same-tile SBUF readers on different
    engines run ~5x slower).
  - Stage 2 accumulates each batch in its own 2-bank PSUM group
    ([num_b|den_b]); per-group epilogue chains stay on the DVE with
    emp1 staged through an idle PSUM bank (a 3-SBUF-stream f32 op runs
    ~1380ns vs ~600ns with one PSUM operand).
  - sigmoid(q)*num/den = num / (den*(1+exp(-q))), so the scalar engine
    only ever evaluates Exp.
"""

import math
from contextlib import ExitStack

import ml_dtypes
import numpy as np

import concourse.bacc as bacc
import concourse.mybir as mybir
import concourse.tile as tile
from concourse.bass_isa import ReduceOp
from concourse.bass_utils import run_bass_kernel_spmd

F32 = mybir.dt.float32
BF16 = mybir.dt.bfloat16
F8 = mybir.dt.float8e4
P = 128

N_CORES = 8
BATCH = 16
N = 2048
D_MODEL = 512

# mybir float8e4 is IEEE-style e4m3: max finite 240, overflow -> inf.
# Keep scaled maxima comfortably below 240 (|u|<~0.75, |ev|<~30, ek<~12).
U_SCALE = 256.0     # fp8 scale for u = exp(pos_bias) - 1
KV_SCALE = 4.0      # fp8 scale for [ev|ek]
INV_SCALE = 1.0 / (U_SCALE * KV_SCALE)
LN_KV_SCALE = math.log(KV_SCALE)


def _install_axon_ntff_shim():
    """Make run_bass_kernel_spmd(trace=True) work when the image's antenv
    lacks axon_hooks (the hook degrades tracing otherwise).  No-op when a
    real antenv.axon_hooks is importable."""
    import sys
    import types

    try:
        import antenv.axon_hooks  # noqa: F401
        return
    except ImportError:
        pass
    try:
        from trn_agent_boot.trn_boot import _ntff_profile_via_ctypes
        hook = _ntff_profile_via_ctypes("/opt/axon/libaxon_pjrt.so")
    except Exception:
        hook = None
    mod = types.ModuleType("antenv.axon_hooks")
    mod.get_axon_ntff_profile_hook = lambda: hook
    mod.set_axon_ntff_profile_hook = lambda h: None
    sys.modules["antenv.axon_hooks"] = mod

    import concourse.bass_utils as bass_utils
    _orig_upload = bass_utils.upload_artifacts

    def _safe_upload(tmpdir):
        try:
            return _orig_upload(tmpdir)
        except Exception:
            return tmpdir
    bass_utils.upload_artifacts = _safe_upload


def build_aft(B=2, N=2048, D=512, n_cores=8, use_bias=False):
    NT = N // P          # row tiles per batch (t / j / i tiles)
    DB = D // P          # d_model blocks of 128 (contraction for projections)
    QKV = 3 * D
    C2 = 2 * B * D       # [ev_b0|ek_b0|ev_b1|ek_b1] column layout
    XW = 8 * P           # x DMA batching: eight t-tiles per transfer
    Exp = mybir.ActivationFunctionType.Exp
    Ident = mybir.ActivationFunctionType.Identity
    Alu = mybir.AluOpType
    DR = mybir.MatmulPerfMode.DoubleRow

    nc = bacc.Bacc("TRN2", target_bir_lowering=False, debug=False,
                   num_devices=n_cores)

    xT_e = nc.dram_tensor("xT", [B, D, N], BF16, kind="ExternalInput")
    x8_e = nc.dram_tensor("x8", [B, D, N], F8, kind="ExternalInput")
    # only v|k in bf16 -- the q projection reads the fp8 wq8 instead, so
    # shipping bf16 Wq would waste 0.5MB of the critical head DMA window
    w_e = nc.dram_tensor("wvk", [D, 2 * D], BF16, kind="ExternalInput")
    wq8_e = nc.dram_tensor("wq8", [P, DB * D], F8, kind="ExternalInput")
    u8_e = nc.dram_tensor("u8", [P, NT, N], F8, kind="ExternalInput")
    if use_bias:
        b_e = nc.dram_tensor("bvkq", [1, QKV], BF16, kind="ExternalInput")
    out_e = nc.dram_tensor("out", [B, N, D], F32, kind="ExternalOutput")

    with tile.TileContext(nc) as tc, ExitStack() as ctx:
        persist = ctx.enter_context(tc.tile_pool(name="persist", bufs=1))
        # psA: [P,1024] granules (2 PSUM banks): stage-1 v|k, stage-2 num|den
        # psB: [P,512] granules (1 bank): stage-1 q, warmup
        psA = ctx.enter_context(tc.tile_pool(name="psA", bufs=3, space="PSUM"))
        psB = ctx.enter_context(tc.tile_pool(name="psB", bufs=2, space="PSUM"))

        # ---- persistent SBUF tensors ----
        u8_sb = persist.tile([P, NT, N], F8)             # 256*(exp(pbT)-1)
        ekv_sb = persist.tile([P, NT, C2], F8)           # 4*[ev|ek] per batch
        q_sb = persist.tile([P, B * NT, D], BF16)        # exp(-q)
        acc_sb = persist.tile([P, C2], F32)              # colsum accumulator
        cs_sb = persist.tile([P, C2], F32)               # all-reduced colsums
        cln4 = persist.tile([P, 1], F32)                 # bias AP: ln(KV_SCALE)
        nc.gpsimd.memset(cln4[:], LN_KV_SCALE)

        with ExitStack() as s1:
            wpool = s1.enter_context(tc.tile_pool(name="wpool", bufs=1))
            kvpool = s1.enter_context(tc.tile_pool(name="kvpool", bufs=6))

            # ---- input DMAs ----
            # x (bf16, for v/k) and x8 (fp8, for the DoubleRow q matmul)
            # live in persistent SBUF tiles.  The leading pieces are small
            # so the first t-tile arrives with minimal transfers; the bulk
            # pieces are paced into the tile loop with a ~4-tile lead so
            # they never crowd the head of the DMA queues (the queues
            # spool up only ~8us after kernel start).
            x_sb = persist.tile([P, DB, B * N], BF16)
            x8_sb = persist.tile([P, DB, B * N], F8)
            xT_r = xT_e.ap().rearrange("b (db p) n -> b p db n", p=P)
            x8_r = x8_e.ap().rearrange("b (db p) n -> b p db n", p=P)

            def emit_x_dma(xb, xt0, xw):
                s0, s1 = (xb * NT + xt0) * P, (xb * NT + xt0 + xw) * P
                nc.sync.dma_start(x_sb[:, :, s0:s1],
                                  xT_r[xb, :, :, xt0 * P:(xt0 + xw) * P])
                nc.sync.dma_start(x8_sb[:, :, s0:s1],
                                  x8_r[xb, :, :, xt0 * P:(xt0 + xw) * P])

            # head pieces: tiles 0-5 of batch 0, interleaved with the
            # weights in consumption order ([v|k] split per d-block so
            # tile 0's first matmuls never wait on a 384KB transfer)
            w_sb = wpool.tile([P, DB, 2 * D], BF16)
            w_r = w_e.ap().rearrange("(db p) c -> db p c", p=P)
            wq_sb = wpool.tile([P, DB, D], F8)
            emit_x_dma(0, 0, 1)
            nc.sync.dma_start(w_sb[:, 0, :D], w_r[0][:, :D])
            emit_x_dma(0, 1, 1)
            nc.sync.dma_start(w_sb[:, 0, D:], w_r[0][:, D:])
            nc.sync.dma_start(wq_sb[:, :, :],
                              wq8_e.ap().rearrange("p (db c) -> p db c", c=D))
            emit_x_dma(0, 2, 2)
            nc.sync.dma_start(w_sb[:, 1, :D], w_r[1][:, :D])
            nc.sync.dma_start(w_sb[:, 1, D:], w_r[1][:, D:])
            emit_x_dma(0, 4, 2)
            for db in range(2, DB):
                nc.sync.dma_start(w_sb[:, db, :D], w_r[db][:, :D])
                nc.sync.dma_start(w_sb[:, db, D:], w_r[db][:, D:])
            if use_bias:
                bias_sb = wpool.tile([1, QKV], BF16)
                nc.sync.dma_start(bias_sb[:], b_e.ap())
                ones_sb = wpool.tile([1, P], BF16)
                nc.vector.memset(ones_sb[:], 1.0)

            # bulk pieces: 2-tile transfers (a 4-tile piece takes too long
            # on its pair of queues and arrives after its consumer).  The
            # early pieces are spread one-per-step (a step-0 burst would
            # starve the head tiles' transfers); later ones keep an
            # 8-tile lead over consumption.
            x_sched = {0: [(0, 6, 2)], 1: [(0, 8, 2)], 2: [(0, 10, 2)],
                       4: [(0, 12, 2)], 6: [(0, 14, 2)]}
            for gidx in range(2 * NT // 2, 2 * NT, 2):
                x_sched.setdefault(gidx - 8, []).append(
                    (gidx // NT, gidx % NT, 2))

            # ---- PE warmup ----
            # The PE clock-gate (HAM) starts at 1.2GHz and releases to
            # 2.4GHz only after ~3.4us of sustained activity.  Issue
            # throwaway matmuls on a memset tile so the clock is warm when
            # the first real projection's x tile lands (~12us in).
            wa = wpool.tile([P, 64], BF16)
            nc.gpsimd.memset(wa[:], 0.0)
            nc.gpsimd.memset(acc_sb[:], 0.0)
            wps = psB.tile([P, D], F32, tag="psB", name="wps")
            for w_i in range(95):
                nc.tensor.matmul(wps[0:64, 0:64], wa[:, 0:64], wa[:],
                                 start=(w_i == 0), stop=(w_i == 94))

            # u8 arrives in four paced 1MB transfers (fp8, host-quantized)
            def emit_u8_dma(part):
                nc.sync.dma_start(u8_sb[:, 4 * part:4 * part + 4, :],
                                  u8_e.ap()[:, 4 * part:4 * part + 4, :])

            # ---- stage 1: projections v/k/q + exp epilogue ----
            step = 0
            for b in range(B):
                for t in range(NT):
                    s = b * NT + t
                    for piece in x_sched.get(step, ()):
                        emit_x_dma(*piece)
                    # pace the u8 stream into the middle of stage 1
                    # (stage 2 only reads u8 ~60us later)
                    if step in (4, 10, 16, 22):
                        emit_u8_dma((step - 4) // 6)

                    ps = psA.tile([P, 2 * D], F32, tag="psA")
                    pq = psB.tile([P, D], F32, tag="psB")
                    for db in range(DB):
                        xt = x_sb[:, db, s * P:(s + 1) * P]
                        st, sp = (db == 0), (db == DB - 1 and not use_bias)
                        nc.tensor.matmul(ps[:, 0:D], xt, w_sb[:, db, 0:D],
                                         start=st, stop=sp)
                        nc.tensor.matmul(ps[:, D:2 * D], xt,
                                         w_sb[:, db, D:2 * D],
                                         start=st, stop=sp)
                    # q = x8 @ wq8 in fp8 DoubleRow (2 passes of 256 rows);
                    # operands are host-scaled by 16 and 8, undone in the
                    # exp(-q/128) epilogue below
                    for h in range(2):
                        nc.tensor.matmul(pq[:, :],
                                         x8_sb[:, 2 * h:2 * h + 2,
                                               s * P:(s + 1) * P],
                                         wq_sb[:, 2 * h:2 * h + 2, :],
                                         start=(h == 0),
                                         stop=(h == 1 and not use_bias),
                                         perf_mode=DR)
                    if use_bias:
                        nc.tensor.matmul(ps[:, 0:D], ones_sb[:, :],
                                         bias_sb[:, 0:D],
                                         start=False, stop=True)
                        nc.tensor.matmul(ps[:, D:2 * D], ones_sb[:, :],
                                         bias_sb[:, D:2 * D],
                                         start=False, stop=True)
                        # q operands are scaled by 128; scale bq to match
                        nc.tensor.matmul(pq[:, :], ones_sb[:, :],
                                         bias_sb[:, 2 * D:],
                                         start=False, stop=True)

                    col = b * 2 * D
                    # bf16 kv = [ev|ek] feeds the f32 colsum + the ev mul
                    kv = kvpool.tile([P, 2 * D], BF16, tag="kv")
                    nc.scalar.activation(kv[:, D:2 * D], ps[:, D:2 * D], Exp)
                    # fp8 ek = 4*exp(k) always on the scalar engine, reading
                    # PSUM: a DVE quantize of kv[D:2D] would overlap the
                    # gpsimd colsum ADD's read of the same kv tile, and that
                    # same-tile SBUF contention slows the DVE op ~5x
                    nc.scalar.activation(
                        ekv_sb[:, t, col + D:col + 2 * D],
                        ps[:, D:2 * D], Exp, bias=cln4[:])
                    # em = exp(-q/128): epilogue uses sigmoid(q)*num/den =
                    # num / (den * (1 + exp(-q)))
                    nc.scalar.activation(q_sb[:, b * NT + t, :], pq[:, :],
                                         Exp, scale=-1.0 / 128.0)
                    nc.vector.tensor_mul(kv[:, 0:D], kv[:, D:2 * D],
                                         ps[:, 0:D])
                    # fp8 ev = 4*ev.  On b1 the DVE also carries the colsum
                    # chain and sits at ~97% occupancy, so alternate b1
                    # tiles hand this quantize to the scalar engine (which
                    # has ~15% slack there; gpsimd is idle on b1 so there
                    # is no same-tile kv contention from it)
                    if b == 1 and t % 2 == 1:
                        nc.scalar.activation(ekv_sb[:, t, col:col + D],
                                             kv[:, 0:D], Ident,
                                             scale=KV_SCALE)
                    else:
                        nc.vector.tensor_scalar_mul(
                            ekv_sb[:, t, col:col + D], kv[:, 0:D], KV_SCALE)
                    # f32 colsum accumulation: serial per-batch chain kept
                    # on ONE engine each (cross-engine chains head-block
                    # the strict FIFOs; gpsimd also reads fp8 ~10x slower
                    # than bf16, so both chains read the bf16 kv):
                    # b0 on gpsimd, b1 on the DVE
                    acc_eng = nc.gpsimd if b == 0 else nc.vector
                    acc_eng.tensor_add(acc_sb[:, col:col + 2 * D],
                                       acc_sb[:, col:col + 2 * D], kv[:])
                    if s == NT:
                        # b0 colsum all-reduce overlapped with b1 stage 1;
                        # its entire input chain also lived on gpsimd, so
                        # this never waits on another engine
                        nc.gpsimd.partition_all_reduce(
                            cs_sb[:, 0:2 * D], acc_sb[:, 0:2 * D],
                            P, ReduceOp.add)
                    step += 1

            nc.gpsimd.partition_all_reduce(
                cs_sb[:, 2 * D:], acc_sb[:, 2 * D:], P, ReduceOp.add)

        # ---- stage 2: num/den contraction over j + epilogue ----
        epi = ctx.enter_context(tc.tile_pool(name="epi", bufs=3))

        for i in range(NT):
            for g in range(B):        # per-batch PSUM group [num_b|den_b]
                # emp1 = 1 + exp(-q) in f32, written into a PSUM bank: the
                # t1 multiply below then reads one PSUM + one SBUF operand,
                # which runs at ~600ns instead of the ~1380ns an all-SBUF
                # f32 three-stream op costs on the DVE.  psB's banks are
                # idle in stage 2.  Emitted before the matmul group so it
                # never sits in the post-matmul chain.
                emp1 = psB.tile([P, D], F32, tag="psB")
                nc.scalar.activation(emp1[:], q_sb[:, g * NT + i, :],
                                     Ident, bias=1.0)

                ps = psA.tile([P, 2 * D], F32, tag="psA")
                for jb2 in range(NT // 2):
                    lhsT = u8_sb[:, 2 * jb2:2 * jb2 + 2, i * P:(i + 1) * P]
                    st, sp = (jb2 == 0), (jb2 == NT // 2 - 1)
                    nc.tensor.matmul(
                        ps[:, 0:D], lhsT,
                        ekv_sb[:, 2 * jb2:2 * jb2 + 2,
                               2 * g * D:(2 * g + 1) * D],
                        start=st, stop=sp, perf_mode=DR)
                    nc.tensor.matmul(
                        ps[:, D:2 * D], lhsT,
                        ekv_sb[:, 2 * jb2:2 * jb2 + 2,
                               (2 * g + 1) * D:(2 * g + 2) * D],
                        start=st, stop=sp, perf_mode=DR)

                col = g * 2 * D
                # num/den = psum/1024 + colsum   (the exact shifted term).
                # The whole per-group chain stays on the DVE: a gpsimd hop
                # inside the chain head-blocks the DVE FIFO (gpsimd muls
                # are ~1.4us) and stalls PSUM recycling.  Only the final
                # o=num*r multiply -- which feeds nothing but the out DMA
                # -- goes to gpsimd (DVE for the last group, whose chain
                # is the exposed kernel tail).  The out store needs no
                # splitting: a dma_start is descriptor-split across all 16
                # queues and moves 256KB in ~2.5us.
                last = (i == NT - 1 and g == B - 1)
                den = epi.tile([P, D], F32, tag="den")
                nc.vector.scalar_tensor_tensor(
                    den[:], ps[:, D:2 * D], INV_SCALE,
                    cs_sb[:, col + D:col + 2 * D],
                    Alu.mult, Alu.add)
                num = epi.tile([P, D], F32, tag="num")
                nc.vector.scalar_tensor_tensor(
                    num[:], ps[:, 0:D], INV_SCALE, cs_sb[:, col:col + D],
                    Alu.mult, Alu.add)
                t1 = epi.tile([P, D], F32, tag="t1")
                nc.vector.tensor_mul(t1[:], emp1[:], den[:])
                r = epi.tile([P, D], F32, tag="r")
                nc.vector.reciprocal_approx_fast(r[:], t1[:])
                o = epi.tile([P, D], F32, tag="o")
                o_eng = nc.vector if last else nc.gpsimd
                o_eng.tensor_mul(o[:], num[:], r[:])
                nc.sync.dma_start(out_e.ap()[g, i * P:(i + 1) * P], o[:])

    nc.compile()
    return nc


_NC_CACHE = {}


def _get_nc(use_bias):
    key = bool(use_bias)
    if key not in _NC_CACHE:
        _NC_CACHE[key] = build_aft(B=BATCH // N_CORES, N=N, D=D_MODEL,
                                   n_cores=N_CORES, use_bias=key)
    return _NC_CACHE[key]


def make_in_maps(x, Wq, bq, Wk, bk, Wv, bv, pos_bias, use_bias):
    """Host-side prep: bf16 casts + the fp8 shifted pos-bias operand in
    stage-2 SBUF layout [jl, jb, i]."""
    NT = N // P
    DB = D_MODEL // P
    Bc = BATCH // N_CORES
    wvk = np.concatenate([Wv, Wk], axis=1).astype(ml_dtypes.bfloat16)
    # q runs in fp8 DoubleRow: host-scale x by 16 and Wq by 8 to lift the
    # operands out of the e4m3 subnormal range; exp(-q/128) undoes it
    wq8 = np.clip(8.0 * Wq, -240, 240).astype(ml_dtypes.float8_e4m3)
    wq8 = np.ascontiguousarray(
        wq8.reshape(DB, P, D_MODEL).transpose(1, 0, 2)).reshape(P, DB * D_MODEL)
    u = U_SCALE * np.expm1(pos_bias.astype(np.float64))       # [i, j]
    u8 = np.clip(u.T, -240.0, 240.0).astype(ml_dtypes.float8_e4m3)  # [j, i]
    u8 = np.ascontiguousarray(
        u8.reshape(NT, P, N).transpose(1, 0, 2))              # [jl, jb, i]
    in_maps = []
    for c in range(N_CORES):
        xT = np.ascontiguousarray(x[c * Bc:(c + 1) * Bc].transpose(0, 2, 1))
        im = {
            "xT": xT.astype(ml_dtypes.bfloat16),
            "x8": np.clip(16.0 * xT, -240, 240).astype(ml_dtypes.float8_e4m3),
            "wvk": wvk,
            "wq8": wq8,
            "u8": u8,
        }
        if use_bias:
            im["bvkq"] = np.concatenate(
                [bv, bk, 128.0 * bq])[None, :].astype(ml_dtypes.bfloat16)
        in_maps.append(im)
    return in_maps


def kernel(x, Wq, bq, Wk, bk, Wv, bv, pos_bias):
    x = np.asarray(x, dtype=np.float32)
    Wq = np.asarray(Wq, dtype=np.float32)
    Wk = np.asarray(Wk, dtype=np.float32)
    Wv = np.asarray(Wv, dtype=np.float32)
    bq = np.asarray(bq, dtype=np.float32)
    bk = np.asarray(bk, dtype=np.float32)
    bv = np.asarray(bv, dtype=np.float32)
    pos_bias = np.asarray(pos_bias, dtype=np.float32)
    assert x.shape == (BATCH, N, D_MODEL)
    assert pos_bias.shape == (N, N)

    _install_axon_ntff_shim()

    use_bias = bool(np.any(bq) or np.any(bk) or np.any(bv))
    nc = _get_nc(use_bias)
    in_maps = make_in_maps(x, Wq, bq, Wk, bk, Wv, bv, pos_bias, use_bias)
    res = run_bass_kernel_spmd(nc, in_maps, core_ids=list(range(N_CORES)))
    out = np.concatenate([res.results[c]["out"] for c in range(N_CORES)],
                         axis=0)
    return out.astype(np.float32, copy=False)
